# revision 1
# baseline (speedup 1.0000x reference)
"""Trainium2 Bass kernel for a pre-norm transformer block (attention + MLP).

Problem: x:[2, 2048, 1024], 16 heads x 64, MLP hidden 4096, fp32.

Sharding: data parallel over tokens. The 4096 tokens are split into 8
blocks of 512 (core c handles batch c//4, sequence block c%4). Each core
recomputes K/V for its whole batch (replicated KV projection -> zero
collectives), runs attention for its own 512 queries over all 2048 keys
of its batch, then the MLP for its own tokens. The host reassembles the
[2, 2048, 1024] output from the 8 per-core [512, 1024] blocks.

Kernel layout strategy (per core):
  - LayerNorm stats are computed token-major [t, c] (free-dim reductions
    via bn_stats), then tiles are PE-transposed to channel-major h^T [c, t]
    which feeds matmuls directly (contraction on partitions).
  - LayerNorm affines are folded into the weights on the host (exact):
    ln_w scales W rows; ln1_b maps to a q bias (the k bias cancels in
    softmax, the v bias folds into proj_b), ln2_b folds into fc1_b.
  - q and k are produced channel-major (q^T, k^T); v is produced
    token-major with an appended ones column so the attention-value
    matmul also accumulates the softmax denominators for free.
  - Scores are computed transposed, S^T[m, t] = k^T.T @ q^T, so the
    attention-value contraction (over keys m) has m on partitions. Two
    heads per matmul slot via row-packed K=64 matmuls (tile_position).
  - Softmax is exp(S/8) without max subtraction (scores are O(+-8) for
    this input distribution - no overflow risk in fp32); the 1/sum
    normalization is applied to the 64-row attention output per head
    instead of the 2048-wide probability matrix.
  - Matmul operands are bf16 (fp32 accumulation in PSUM); the residual
    stream, LayerNorm statistics and all PSUM math stay fp32, keeping
    the end-to-end relative error ~1e-3. ACT_BF16=False falls back to
    float32r operands (~1e-4) at 2x the weight-DMA cost.
"""

import numpy as np
from contextlib import ExitStack

import concourse.bass as bass
import concourse.tile as tile
from concourse import mybir
from concourse.bass_utils import run_bass_kernel_spmd
from concourse.masks import make_identity

FP32 = mybir.dt.float32
FP32R = mybir.dt.float32r
BF16 = mybir.dt.bfloat16
AF = mybir.ActivationFunctionType
ALU = mybir.AluOpType

N_CORES = 8
B, N, C, H, D, F = 2, 2048, 1024, 16, 64, 4096
T = 512            # tokens owned per core
M = 2048           # keys (full batch sequence)
EPS = 1e-5
SCALE = float(D) ** -0.5   # 0.125

CB = C // 128      # 8 channel blocks
TB = T // 128      # 4 own-token blocks
MI = M // 128      # 16 key 128-chunks
MC = M // 512      # 4 key 512-chunks
FB = F // 128      # 32 mlp hidden blocks
NG = 4             # head groups
GP = 2             # head pairs per group (4 heads per group)

# tuning knobs (module-level so experiments can flip them before build())
ACT_BF16 = True    # bf16 matmul operands + weights (else float32r)
KV_ALLGATHER = True  # shard K/V projection and all-gather within batch groups
SKIP_CC = False      # timing-only: emit without collectives (wrong results)
SAB_BUFS = 2       # buffers for the score psum tag
STOP_AFTER = ""    # debug: stop emission after a phase name


def r32(ap):
    return ap.bitcast(FP32R)


def _ln_norm(nc, work, x_sb, xn_sb, eps_sb):
    """xn = (x - mean(x)) * rsqrt(var(x) + eps) along the free dim (1024)."""
    stats = work.tile([128, 2, 6], FP32, name="ln_stats")
    nc.vector.bn_stats(out=stats[:, 0, :], in_=x_sb[:, 0:512])
    nc.vector.bn_stats(out=stats[:, 1, :], in_=x_sb[:, 512:1024])
    mv = work.tile([128, 2], FP32, name="ln_mv")
    nc.vector.bn_aggr(out=mv, in_=stats)
    sd = work.tile([128, 1], FP32, name="ln_sd")
    nc.scalar.activation(out=sd, in_=mv[:, 1:2], func=AF.Sqrt, bias=eps_sb, scale=1.0)
    rsig = work.tile([128, 1], FP32, name="ln_rsig")
    nc.vector.reciprocal(out=rsig, in_=sd)
    nc.vector.tensor_scalar(
        out=xn_sb, in0=x_sb, scalar1=mv[:, 0:1], scalar2=rsig,
        op0=ALU.subtract, op1=ALU.mult,
    )


def _emit_attn_ag(ctx, tc, io, nc, ADT, wc, qkv_r, xown_r2, qT, oT,
                  qb_sb, ones, eps_sb, ident):
    """AllGather variant: LN1 + q/k/v over OWN tokens only, all-gather k/v
    (bf16, ~2MB/core) within the 4-core batch group, then attention over
    the gathered keys/values."""
    with (
        tc.tile_pool(name="ag_h", bufs=1) as p_h,
        tc.tile_pool(name="ag_dram", bufs=1, space="DRAM") as p_dram,
    ):
        hT = p_h.tile([128, CB, T], ADT)
        with (
            tc.tile_pool(name="ln1_work", bufs=4) as w1,
            tc.tile_pool(name="ln1_ps", bufs=4, space="PSUM") as ps_t,
        ):
            for tb in range(TB):
                xc = w1.tile([128, C], FP32, name="ln1_x")
                nc.sync.dma_start(out=xc, in_=xown_r2[tb])
                xn = w1.tile([128, C], FP32, name="ln1_xn")
                _ln_norm(nc, w1, xc, xn, eps_sb)
                for h4 in range(2):
                    tp = ps_t.tile([128, 4, 128], FP32, name="ln1_tp")
                    for j in range(4):
                        cb = h4 * 4 + j
                        nc.tensor.transpose(
                            tp[:, j, :], xn[:, cb * 128:(cb + 1) * 128], ident)
                    nc.vector.tensor_copy(
                        out=hT[:, h4 * 4:(h4 + 1) * 4, tb * 128:(tb + 1) * 128],
                        in_=tp)

        if STOP_AFTER == "ln1":
            return

        k_in = p_dram.tile([C, T], ADT, name="k_in")
        v_in = p_dram.tile([T, C], ADT, name="v_in")
        k_out = p_dram.tile([4, C, T], ADT, name="k_out")
        v_out = p_dram.tile([4, T, C], ADT, name="v_out")

        with (
            tc.tile_pool(name="ag_qkvw", bufs=2) as qkvw,
            tc.tile_pool(name="ag_kv", bufs=1) as p_kv,
            tc.tile_pool(name="ag_ps", bufs=3, space="PSUM") as ps_q,
        ):
            # k projection (own tokens), channel-major
            k_sb = p_kv.tile([128, CB, T], ADT, name="k_sb")
            for kb in range(CB):
                wk = qkvw.tile([128, CB, 128], ADT, name="wk")
                nc.sync.dma_start(
                    out=wk,
                    in_=wc(qkv_r[:, :, C + kb * 128:C + (kb + 1) * 128]))
                pk = ps_q.tile([128, T], FP32, name="pq")
                for cb in range(CB):
                    nc.tensor.matmul(
                        pk, wk[:, cb, :], hT[:, cb, :],
                        start=(cb == 0), stop=(cb == CB - 1))
                nc.vector.tensor_copy(out=k_sb[:, kb, :], in_=pk)
                nc.sync.dma_start(
                    out=k_in.rearrange("(kb p) t -> p kb t", p=128)[:, kb],
                    in_=k_sb[:, kb, :])

            # v projection (own tokens), token-major
            wv0 = qkvw.tile([128, CB, T], ADT, name="wv0")
            nc.sync.dma_start(
                out=wv0, in_=wc(qkv_r[:, :, 2 * C:2 * C + 512]))
            wv1 = qkvw.tile([128, CB, T], ADT, name="wv1")
            nc.sync.dma_start(
                out=wv1, in_=wc(qkv_r[:, :, 2 * C + 512:3 * C]))
            v_sb = p_kv.tile([128, TB, C], ADT, name="v_sb")
            for tb in range(TB):
                for vc, wv in ((0, wv0), (1, wv1)):
                    pv = ps_q.tile([128, T], FP32, name="pq")
                    for cb in range(CB):
                        nc.tensor.matmul(
                            pv, hT[:, cb, tb * 128:(tb + 1) * 128],
                            wv[:, cb, :],
                            start=(cb == 0), stop=(cb == CB - 1))
                    nc.vector.tensor_copy(
                        out=v_sb[:, tb, vc * 512:(vc + 1) * 512], in_=pv)
                nc.sync.dma_start(
                    out=v_in.rearrange("(tb p) c -> p tb c", p=128)[:, tb],
                    in_=v_sb[:, tb, :])

            # all-gather k and v within the 4-core batch group
            if not SKIP_CC:
                groups = [[0, 1, 2, 3], [4, 5, 6, 7]]
                nc.gpsimd.collective_compute(
                    "AllGather", ALU.bypass, replica_groups=groups,
                    ins=[k_in.opt()], outs=[k_out.opt()])
                nc.gpsimd.collective_compute(
                    "AllGather", ALU.bypass, replica_groups=groups,
                    ins=[v_in.opt()], outs=[v_out.opt()])
            else:
                for s in range(4):
                    nc.sync.dma_start(out=k_out[s], in_=k_in[:, :])
                    nc.sync.dma_start(out=v_out[s], in_=v_in[:, :])

            # q projection (own tokens) - overlaps the gather
            for qb in range(CB):
                wq = qkvw.tile([128, CB, 128], ADT, name="wk")
                nc.sync.dma_start(
                    out=wq, in_=wc(qkv_r[:, :, qb * 128:(qb + 1) * 128]))
                pq = ps_q.tile([128, T], FP32, name="pq")
                for cb in range(CB):
                    nc.tensor.matmul(
                        pq, wq[:, cb, :], hT[:, cb, :],
                        start=(cb == 0), stop=(cb == CB - 1))
                nc.vector.tensor_scalar(
                    out=qT[:, qb, :], in0=pq,
                    scalar1=qb_sb[:, qb:qb + 1], scalar2=None,
                    op0=ALU.add)
        if STOP_AFTER == "qproj":
            return

        # ---- load gathered k/v and run the flat 8-pair attention ----
        with (
            tc.tile_pool(name="ag_kt", bufs=1) as p_kt,
            tc.tile_pool(name="a_p", bufs=2) as pp,
            tc.tile_pool(name="a_r", bufs=1) as pr,
            tc.tile_pool(name="a_pss", bufs=SAB_BUFS, space="PSUM") as ps_s,
            tc.tile_pool(name="a_pso", bufs=2, space="PSUM") as ps_o,
        ):
            kT = p_kt.tile([128, CB, 4, T], ADT, name="kTa")
            k_out_r = k_out.rearrange("s (kb p) t -> p kb s t", p=128)
            for kb in range(CB):
                nc.sync.dma_start(out=kT[:, kb], in_=k_out_r[:, kb])
            vg = p_kt.tile([128, MI, H, D + 1], ADT, name="vga")
            ones_col = bass.AP(
                tensor=ones.tensor, offset=ones[:, 0:1].offset,
                ap=[ones.ap[0], [0, MI], [0, H], [1, 1]])
            nc.vector.tensor_copy(out=vg[:, :, :, D:D + 1], in_=ones_col)
            v_out_r = v_out.rearrange("s (lc p) (h d) -> p s lc h d",
                                      p=128, d=D)
            for s in range(4):
                for lc in range(TB):
                    nc.sync.dma_start(out=vg[:, s * TB + lc, :, 0:D],
                                      in_=v_out_r[:, s, lc])

            for pair in range(H // 2):
                hA, hB = 2 * pair, 2 * pair + 1
                oA = ps_o.tile([128, T], FP32, name="oA")
                oB = ps_o.tile([128, T], FP32, name="oB")
                for mi in range(MI):
                    s, lc = mi // 4, mi % 4
                    msl = slice(lc * 128, (lc + 1) * 128)
                    sAB = ps_s.tile([128, 2, T], FP32, name="sAB")
                    nc.tensor.matmul(
                        sAB[:, 0, :], kT[0:64, pair, s, msl],
                        qT[0:64, pair, :], start=True, stop=True)
                    nc.tensor.matmul(
                        sAB[:, 1, :], kT[64:128, pair, s, msl],
                        qT[64:128, pair, :], start=True, stop=True)
                    pAB = pp.tile([128, 2, T], ADT, name="pAB")
                    nc.scalar.activation(out=pAB, in_=sAB, func=AF.Exp,
                                         scale=SCALE)
                    nc.tensor.matmul(
                        oA[0:D + 1, :], vg[:, mi, hA, :], pAB[:, 0, :],
                        start=(mi == 0), stop=(mi == MI - 1))
                    nc.tensor.matmul(
                        oB[0:D + 1, :], vg[:, mi, hB, :], pAB[:, 1, :],
                        start=(mi == 0), stop=(mi == MI - 1))

                rec = pr.tile([128, T], FP32, name="rec")
                nc.vector.reciprocal(out=rec[64:65, :], in_=oA[64:65, :])
                recr = pr.tile([128, T], ADT, name="recr")
                nc.vector.tensor_copy(out=recr[64:65, :], in_=rec[64:65, :])
                rbA_ps = ps_s.tile([128, 2, T], FP32, name="sAB")[:, 0, :]
                nc.tensor.matmul(
                    rbA_ps[0:64, :], ones[64:65, 0:64], recr[64:65, :],
                    start=True, stop=True)
                rbA = pr.tile([128, T], FP32, name="rbA")
                nc.vector.tensor_copy(out=rbA[0:64, :], in_=rbA_ps[0:64, :])
                rec2 = pr.tile([128, T], FP32, name="rec2")
                nc.vector.reciprocal(out=rec2[64:65, :], in_=oB[64:65, :])
                rec2r = pr.tile([128, T], ADT, name="rec2r")
                nc.vector.tensor_copy(out=rec2r[64:65, :], in_=rec2[64:65, :])
                rbB_ps = ps_s.tile([128, 2, T], FP32, name="sAB")[:, 1, :]
                nc.tensor.matmul(
                    rbB_ps[0:64, :], ones[64:65, 0:64], rec2r[64:65, :],
                    start=True, stop=True)
                rbB = pr.tile([128, T], FP32, name="rbB")
                nc.vector.tensor_copy(out=rbB[0:64, :], in_=rbB_ps[0:64, :])
                nc.vector.tensor_mul(
                    out=oT[0:64, pair, :], in0=oA[0:64, :], in1=rbA[0:64, :])
                tmpB = pr.tile([128, T], ADT, name="tmpB")
                nc.vector.tensor_mul(
                    out=tmpB[0:64, :], in0=oB[0:64, :], in1=rbB[0:64, :])
                nc.sync.dma_start(
                    out=oT[64:128, pair, :], in_=tmpB[0:64, :])



def _emit(ctx: ExitStack, tc: tile.TileContext, io: dict):
    nc = tc.nc
    ADT = BF16 if ACT_BF16 else FP32R      # matmul operand dtype
    wc = (lambda ap: ap) if ACT_BF16 else r32   # weight AP cast

    xb = io["xb"]          # [2048, 1024] full batch rows
    xown = io["xown"]      # [512, 1024] own rows
    qkv_w = io["qkv_w"]    # [1024, 3072] (ln1_w folded in)
    proj_w = io["proj_w"]  # [1024, 1024]
    proj_b = io["proj_b"]  # [1024] (+ folded v bias)
    q_bias = io["q_bias"]  # [1024] folded ln1_b @ Wq
    fc1_w, fc1_b = io["fc1_w"], io["fc1_b"]   # ln2 folded in
    fc2_w, fc2_b = io["fc2_w"], io["fc2_b"]
    y = io["y"]            # [512, 1024] output

    xb_r = xb.rearrange("(mi p) c -> mi p c", p=128)          # [16, 128, 1024]
    xown_r2 = xown.rearrange("(tb p) c -> tb p c", p=128)     # [4, 128, 1024]
    xown_r = xown.rearrange("(tb p) c -> p tb c", p=128)      # [128, 4, 1024]
    qkv_r = qkv_w.rearrange("(cb p) o -> p cb o", p=128)      # [128, 8, 3072]
    proj_r = proj_w.rearrange("(cb p) o -> p cb o", p=128)    # [128, 8, 1024]
    fc1_r = fc1_w.rearrange("(cb p) f -> p cb f", p=128)      # [128, 8, 4096]
    fc2_r = fc2_w.rearrange("(fb p) c -> p fb c", p=128)      # [128, 32, 1024]
    y_r = y.rearrange("(tb p) c -> p tb c", p=128)            # [128, 4, 1024]

    # --- constants (live whole kernel) ---
    consts = ctx.enter_context(tc.tile_pool(name="consts", bufs=1))

    ident = consts.tile([128, 128], FP32)
    make_identity(nc, ident)
    ones_f = consts.tile([128, 128], FP32)
    nc.vector.memset(ones_f, 1.0)
    ones = consts.tile([128, 128], ADT)
    nc.vector.tensor_copy(out=ones, in_=ones_f)
    eps_sb = consts.tile([128, 1], FP32)
    nc.vector.memset(eps_sb, EPS)

    def load_vec_pcb(vec, nblk, name):
        t = consts.tile([128, nblk], FP32, name=name)
        nc.sync.dma_start(out=t, in_=vec.rearrange("(b p) -> p b", p=128))
        return t

    qb_sb = load_vec_pcb(q_bias, CB, "qb")
    fc1b_sb = load_vec_pcb(fc1_b, FB, "fc1b")

    def bcast_rows_pool(pool, vec, name):
        t = pool.tile([128, C], FP32, name=name)
        src = bass.AP(tensor=vec.tensor, offset=vec.offset, ap=[[0, 128]] + vec.ap)
        nc.sync.dma_start(out=t, in_=src)
        return t

    # --- x2 / h2T: carried from attention into the MLP ---
    p_mid = ctx.enter_context(tc.tile_pool(name="p_mid", bufs=1))
    x2 = p_mid.tile([128, TB, C], FP32)      # residual stream after attention
    h2T = p_mid.tile([128, CB, T], ADT)      # LN2 output, channel-major

    # --- qT; its pair slots are reused for oT after the pair's scores ---
    p_attn = ctx.enter_context(tc.tile_pool(name="p_attn", bufs=1))
    qT = p_attn.tile([128, CB, T], ADT)
    oT = qT                                   # alias: oT[:, pair] overwrites
    #                                           qT[:, pair] after last use

    if KV_ALLGATHER:
        _emit_attn_ag(ctx, tc, io, nc, ADT, wc, qkv_r, xown_r2, qT, oT,
                      qb_sb, ones, eps_sb, ident)
    else:
        with tc.tile_pool(name="p_h1", bufs=1) as p_h1:
            h1T = p_h1.tile([128, CB, M], ADT)

            # -----------------------------------------------------------
            # Phase 1: LN1 -> h1T [c, m] (batch); hqT [c, t] (own tokens)
            # -----------------------------------------------------------
            with tc.tile_pool(name="p_hq", bufs=1) as p_hq:
                hqT = p_hq.tile([128, CB, T], ADT)

                with (
                    tc.tile_pool(name="ln1_work", bufs=3) as w1,
                    tc.tile_pool(name="ln1_ps", bufs=4, space="PSUM") as ps_t,
                ):
                    def ln_block(src_ap, dstT, dst_col):
                        xc = w1.tile([128, C], FP32, name="ln1_x")
                        nc.sync.dma_start(out=xc, in_=src_ap)
                        xn = w1.tile([128, C], FP32, name="ln1_xn")
                        _ln_norm(nc, w1, xc, xn, eps_sb)
                        for h4 in range(2):      # 4 transposes -> 1 drain copy
                            tp = ps_t.tile([128, 4, 128], FP32, name="ln1_tp")
                            for j in range(4):
                                cb = h4 * 4 + j
                                nc.tensor.transpose(
                                    tp[:, j, :], xn[:, cb * 128:(cb + 1) * 128],
                                    ident)
                            nc.vector.tensor_copy(
                                out=dstT[:, h4 * 4:(h4 + 1) * 4,
                                         dst_col:dst_col + 128],
                                in_=tp)

                    for mi in range(MI):
                        ln_block(xb_r[mi], h1T, mi * 128)
                    for tb in range(TB):
                        ln_block(xown_r2[tb], hqT, tb * 128)

                if STOP_AFTER == "ln1":
                    return
                # -------------------------------------------------------
                # Phase 2: Q projection -> qT [qdim, t] (own tokens)
                # -------------------------------------------------------
                with (
                    tc.tile_pool(name="q_w", bufs=2) as qw_pool,
                    tc.tile_pool(name="q_ps", bufs=3, space="PSUM") as ps_q,
                ):
                    for qb in range(CB):
                        wq = qw_pool.tile([128, CB, 128], ADT, name="wq")
                        nc.sync.dma_start(
                            out=wq, in_=wc(qkv_r[:, :, qb * 128:(qb + 1) * 128]))
                        pq = ps_q.tile([128, T], FP32, name="pq")
                        for cb in range(CB):
                            nc.tensor.matmul(
                                pq, wq[:, cb, :], hqT[:, cb, :],
                                start=(cb == 0), stop=(cb == CB - 1),
                            )
                        nc.vector.tensor_scalar(
                            out=qT[:, qb, :], in0=pq,
                            scalar1=qb_sb[:, qb:qb + 1], scalar2=None,
                            op0=ALU.add)
                if STOP_AFTER == "qproj":
                    return

            # -----------------------------------------------------------
            # Phase 3: attention, 4 head-groups of 4 heads (2 pairs each)
            # -----------------------------------------------------------
            with (
                tc.tile_pool(name="a_w", bufs=2) as gw,
                tc.tile_pool(name="a_big", bufs=2) as gbig,
                tc.tile_pool(name="a_p", bufs=2) as pp,
                tc.tile_pool(name="a_r", bufs=1) as pr,
                tc.tile_pool(name="a_pskv", bufs=1, space="PSUM") as ps_kv,
                tc.tile_pool(name="a_pss", bufs=SAB_BUFS, space="PSUM") as ps_s,
                tc.tile_pool(name="a_pso", bufs=1, space="PSUM") as ps_o,
            ):
                for g in range(NG):
                    # K/V projections for the whole group first. Pools are
                    # shared across groups (bufs=2 on kT/vg) so group g+1's
                    # projections overlap group g's attention pair loop.
                    kT = gbig.tile([128, GP, M], ADT, name="kT")
                    vg = gbig.tile([128, MI, 2 * GP, D + 1], ADT, name="vg")
                    for pb in range(GP):
                        pair = g * GP + pb        # global pair idx
                        wk = gw.tile([128, CB, 128], ADT, name="wk")
                        nc.sync.dma_start(
                            out=wk,
                            in_=wc(qkv_r[:, :, C + pair * 128:
                                         C + (pair + 1) * 128]))
                        for mc in range(MC):
                            pk = ps_kv.tile([128, 512], FP32, name="psk")
                            for cb in range(CB):
                                nc.tensor.matmul(
                                    pk, wk[:, cb, :],
                                    h1T[:, cb, mc * 512:(mc + 1) * 512],
                                    start=(cb == 0), stop=(cb == CB - 1),
                                )
                            nc.vector.tensor_copy(
                                out=kT[:, pb, mc * 512:(mc + 1) * 512], in_=pk)

                    # V: token-major + ones column for the softmax sums
                    ones_col = bass.AP(
                        tensor=ones.tensor, offset=ones[:, 0:1].offset,
                        ap=[ones.ap[0], [0, MI], [0, 2 * GP], [1, 1]])
                    nc.vector.tensor_copy(out=vg[:, :, :, D:D + 1],
                                          in_=ones_col)
                    wv = gw.tile([128, CB, 2 * GP * D], ADT, name="wv")
                    nc.sync.dma_start(
                        out=wv,
                        in_=wc(qkv_r[:, :, 2 * C + g * 256:
                                     2 * C + (g + 1) * 256]))
                    for mi in range(MI):
                        pv = ps_kv.tile([128, 256], FP32, name="psv")
                        for cb in range(CB):
                            nc.tensor.matmul(
                                pv, h1T[:, cb, mi * 128:(mi + 1) * 128],
                                wv[:, cb, :],
                                start=(cb == 0), stop=(cb == CB - 1),
                            )
                        nc.vector.tensor_copy(
                            out=vg[:, mi, :, 0:D],
                            in_=pv.rearrange("p (h d) -> p h d", d=D))

                    for pb in range(GP):
                        pair = g * GP + pb
                        hA, hB = 2 * pb, 2 * pb + 1     # local head idx in group
                        oA = ps_o.tile([128, T], FP32, name="oA")   # rows 0:65
                        oB = ps_o.tile([128, T], FP32, name="oB")   # rows 0:65
                        for mi in range(MI):
                            msl = slice(mi * 128, (mi + 1) * 128)
                            sAB = ps_s.tile([128, 2, T], FP32, name="sAB")
                            # scores (transposed): S^T[m, t], row-packed pair
                            nc.tensor.matmul(
                                sAB[:, 0, :], kT[0:64, pb, msl], qT[0:64, pair, :],
                                start=True, stop=True)
                            nc.tensor.matmul(
                                sAB[:, 1, :], kT[64:128, pb, msl],
                                qT[64:128, pair, :],
                                start=True, stop=True)
                            pAB = pp.tile([128, 2, T], ADT, name="pAB")
                            nc.scalar.activation(out=pAB, in_=sAB, func=AF.Exp,
                                                 scale=SCALE)
                            # attention @ V; ones column accumulates softmax sums
                            nc.tensor.matmul(
                                oA[0:D + 1, :], vg[:, mi, hA, :], pAB[:, 0, :],
                                start=(mi == 0), stop=(mi == MI - 1))
                            nc.tensor.matmul(
                                oB[0:D + 1, :], vg[:, mi, hB, :], pAB[:, 1, :],
                                start=(mi == 0), stop=(mi == MI - 1))

                        # normalize: oT[head] = o_unnorm * (1/sums), broadcast
                        # over d via a ones-outer-product matmul. Matmul outputs
                        # must start at partition 0, so head B is normalized into
                        # a temp tile and partition-shifted to rows 64:128 of oT
                        # with an SBUF->SBUF DMA. (oT aliases qT: the pair slot
                        # is dead after this pair's scores.)
                        rec = pr.tile([128, T], FP32, name="rec")
                        nc.vector.reciprocal(out=rec[64:65, :], in_=oA[64:65, :])
                        recr = pr.tile([128, T], ADT, name="recr")
                        nc.vector.tensor_copy(out=recr[64:65, :], in_=rec[64:65, :])
                        rbA_ps = ps_s.tile([128, 2, T], FP32, name="sAB")[:, 0, :]
                        nc.tensor.matmul(
                            rbA_ps[0:64, :], ones[64:65, 0:64], recr[64:65, :],
                            start=True, stop=True)
                        rbA = pr.tile([128, T], FP32, name="rbA")
                        nc.vector.tensor_copy(out=rbA[0:64, :], in_=rbA_ps[0:64, :])
                        rec2 = pr.tile([128, T], FP32, name="rec2")
                        nc.vector.reciprocal(out=rec2[64:65, :], in_=oB[64:65, :])
                        rec2r = pr.tile([128, T], ADT, name="rec2r")
                        nc.vector.tensor_copy(out=rec2r[64:65, :],
                                              in_=rec2[64:65, :])
                        rbB_ps = ps_s.tile([128, 2, T], FP32, name="sAB")[:, 1, :]
                        nc.tensor.matmul(
                            rbB_ps[0:64, :], ones[64:65, 0:64], rec2r[64:65, :],
                            start=True, stop=True)
                        rbB = pr.tile([128, T], FP32, name="rbB")
                        nc.vector.tensor_copy(out=rbB[0:64, :], in_=rbB_ps[0:64, :])
                        nc.vector.tensor_mul(
                            out=oT[0:64, pair, :], in0=oA[0:64, :], in1=rbA[0:64, :])
                        tmpB = pr.tile([128, T], ADT, name="tmpB")
                        nc.vector.tensor_mul(
                            out=tmpB[0:64, :], in0=oB[0:64, :], in1=rbB[0:64, :])
                        nc.sync.dma_start(
                            out=oT[64:128, pair, :], in_=tmpB[0:64, :])

    if STOP_AFTER == "attn":
        return
    # ---------------------------------------------------------------
    # Phase 4: attention projection + residual -> x2 [t, c] token-major
    # ---------------------------------------------------------------
    with (
        tc.tile_pool(name="pj_w", bufs=2) as pjw,
        tc.tile_pool(name="pj_work", bufs=1) as pjwork,
        tc.tile_pool(name="pj_ps", bufs=1, space="PSUM") as ps_pj,
    ):
        b1bc = bcast_rows_pool(pjwork, proj_b, "b1bc")
        xob = pjwork.tile([128, TB, C], FP32, name="xob")
        nc.sync.dma_start(out=xob, in_=xown_r)
        for tb in range(TB):
            nc.vector.tensor_add(out=xob[:, tb, :], in0=xob[:, tb, :], in1=b1bc)
        # psum tiles held across the cpb accumulation: 8 banks exactly
        ppj = {}
        for tb in range(TB):
            for cc in range(2):
                ppj[(tb, cc)] = ps_pj.tile([128, 512], FP32, name=f"ppj_{tb}_{cc}")
        for cpb in range(CB):
            wpj = pjw.tile([128, C], ADT, name="wpj")
            nc.sync.dma_start(out=wpj, in_=wc(proj_r[:, cpb, :]))
            for tb in range(TB):
                for cc in range(2):
                    nc.tensor.matmul(
                        ppj[(tb, cc)], oT[:, cpb, tb * 128:(tb + 1) * 128],
                        wpj[:, cc * 512:(cc + 1) * 512],
                        start=(cpb == 0), stop=(cpb == CB - 1),
                    )
        for tb in range(TB):
            for cc in range(2):
                nc.vector.tensor_add(
                    out=x2[:, tb, cc * 512:(cc + 1) * 512],
                    in0=ppj[(tb, cc)], in1=xob[:, tb, cc * 512:(cc + 1) * 512])

    if STOP_AFTER == "proj":
        return
    # ---------------------------------------------------------------
    # Phase 5: LN2 -> h2T [c, t]; then x2 += fc2 bias (residual base)
    # ---------------------------------------------------------------
    with (
        tc.tile_pool(name="ln2_work", bufs=3) as w2,
        tc.tile_pool(name="ln2_ps", bufs=4, space="PSUM") as ps_t2,
    ):
        for tb in range(TB):
            xn = w2.tile([128, C], FP32, name="ln2_xn")
            _ln_norm(nc, w2, x2[:, tb, :], xn, eps_sb)
            for h4 in range(2):
                tp = ps_t2.tile([128, 4, 128], FP32, name="ln2_tp")
                for j in range(4):
                    cb = h4 * 4 + j
                    nc.tensor.transpose(
                        tp[:, j, :], xn[:, cb * 128:(cb + 1) * 128], ident)
                nc.vector.tensor_copy(
                    out=h2T[:, h4 * 4:(h4 + 1) * 4, tb * 128:(tb + 1) * 128],
                    in_=tp)
    b2bc = bcast_rows_pool(p_mid, fc2_b, "b2bc")
    for tb in range(TB):
        nc.vector.tensor_add(out=x2[:, tb, :], in0=x2[:, tb, :], in1=b2bc)

    if STOP_AFTER == "ln2":
        return
    # ---------------------------------------------------------------
    # Phase 6: MLP fc1 (gelu) -> gT [f, t]; fc2 + residual -> y
    # ---------------------------------------------------------------
    with (
        tc.tile_pool(name="p_g", bufs=1) as p_g,
        tc.tile_pool(name="f_w", bufs=2) as fw,
        tc.tile_pool(name="f_out", bufs=4) as fout,
    ):
        gT = p_g.tile([128, FB, T], ADT)

        with tc.tile_pool(name="f1_ps", bufs=3, space="PSUM") as ps_f1:
            for fq in range(FB // 4):
                w1t = fw.tile([128, CB, 512], ADT, name="w1t")
                nc.sync.dma_start(
                    out=w1t, in_=wc(fc1_r[:, :, fq * 512:(fq + 1) * 512]))
                for j in range(4):
                    fb = fq * 4 + j
                    pf = ps_f1.tile([128, T], FP32, name="pf")
                    for cb in range(CB):
                        nc.tensor.matmul(
                            pf, w1t[:, cb, j * 128:(j + 1) * 128],
                            h2T[:, cb, :],
                            start=(cb == 0), stop=(cb == CB - 1),
                        )
                    nc.scalar.activation(
                        out=gT[:, fb, :], in_=pf, func=AF.Gelu,
                        bias=fc1b_sb[:, fb:fb + 1], scale=1.0,
                    )

        # fc2: all 8 [t, c] psum accumulators live at once (8 banks; the
        # fc1 psum pool is closed), so each weight tile streams exactly once.
        with tc.tile_pool(name="f2_ps", bufs=1, space="PSUM") as ps_f2:
            held = {}
            for tb in range(TB):
                for cc in range(2):
                    held[(tb, cc)] = ps_f2.tile(
                        [128, 512], FP32, name=f"pf2_{tb}_{cc}")
            for fb in range(FB):
                w2t = fw.tile([128, C], ADT, name="w2t")
                nc.sync.dma_start(out=w2t, in_=wc(fc2_r[:, fb, :]))
                for tb in range(TB):
                    for cc in range(2):
                        nc.tensor.matmul(
                            held[(tb, cc)], gT[:, fb, tb * 128:(tb + 1) * 128],
                            w2t[:, cc * 512:(cc + 1) * 512],
                            start=(fb == 0), stop=(fb == FB - 1),
                        )
            for tb in range(TB):
                for cc in range(2):
                    yt = fout.tile([128, 512], FP32, name="yt")
                    nc.vector.tensor_add(
                        out=yt, in0=held[(tb, cc)],
                        in1=x2[:, tb, cc * 512:(cc + 1) * 512])
                    nc.sync.dma_start(
                        out=y_r[:, tb, cc * 512:(cc + 1) * 512], in_=yt)


def split_excess_waits(nc, limit=1):
    """This walrus build only supports ONE sync wait per engine instruction.
    Move excess waits onto NOPs inserted just before the instruction on the
    same engine (for DMAs, move all waits so the descriptor carries none)."""
    for f in nc.m.functions:
        for bb in f.blocks:
            new_insts = []
            for inst in bb.instructions:
                si = getattr(inst, "sync_info", None)
                if si is not None and si.on_wait and len(si.on_wait) > limit:
                    waits = list(si.on_wait)
                    if isinstance(inst, mybir.InstDMACopy):
                        moved, si.on_wait = waits, []
                    else:
                        moved, si.on_wait = waits[limit:], waits[:limit]
                    for j, w in enumerate(moved):
                        nop = mybir.InstNoOp(
                            name=f"{inst.name}-xw{j}",
                            engine=inst.engine,
                            sync_info=mybir.SyncInfo(on_wait=[w], on_update=[]),
                            bass_nofuse=True,
                        )
                        new_insts.append(nop)
                new_insts.append(inst)
            bb.instructions[:] = new_insts


_CACHE = {}


def build():
    key = (ACT_BF16, SAB_BUFS, STOP_AFTER, KV_ALLGATHER, SKIP_CC)
    if key in _CACHE:
        return _CACHE[key]

    nc = bass.Bass("TRN2", target_bir_lowering=False, debug=False,
                   num_devices=N_CORES)
    wdt = BF16 if ACT_BF16 else FP32
    io = {}
    io["xb"] = nc.dram_tensor("xb", [M, C], FP32, kind="ExternalInput").ap()
    io["xown"] = nc.dram_tensor("xown", [T, C], FP32, kind="ExternalInput").ap()
    io["qkv_w"] = nc.dram_tensor("qkv_w", [C, 3 * C], wdt, kind="ExternalInput").ap()
    io["proj_w"] = nc.dram_tensor("proj_w", [C, C], wdt, kind="ExternalInput").ap()
    io["proj_b"] = nc.dram_tensor("proj_b", [C], FP32, kind="ExternalInput").ap()
    io["q_bias"] = nc.dram_tensor("q_bias", [C], FP32, kind="ExternalInput").ap()
    io["fc1_w"] = nc.dram_tensor("fc1_w", [C, F], wdt, kind="ExternalInput").ap()
    io["fc1_b"] = nc.dram_tensor("fc1_b", [F], FP32, kind="ExternalInput").ap()
    io["fc2_w"] = nc.dram_tensor("fc2_w", [F, C], wdt, kind="ExternalInput").ap()
    io["fc2_b"] = nc.dram_tensor("fc2_b", [C], FP32, kind="ExternalInput").ap()
    io["y"] = nc.dram_tensor("y", [T, C], FP32, kind="ExternalOutput").ap()

    with tile.TileContext(nc) as tc:
        with ExitStack() as ctx:
            _emit(ctx, tc, io)

    split_excess_waits(nc)
    _CACHE[key] = nc
    return nc


def make_in_maps(inputs):
    x = np.ascontiguousarray(np.asarray(inputs["x"]), dtype=np.float32)
    f64 = {k: np.asarray(inputs[k], dtype=np.float64)
           for k in ("qkv_w", "proj_w", "proj_b", "ln1_w", "ln1_b", "ln2_w",
                     "ln2_b", "fc1_w", "fc1_b", "fc2_w", "fc2_b")}
    # Fold LayerNorm affines into the weights (exact up to fp32 rounding):
    #   h = xn*ln_w + ln_b;  h @ W = xn @ (ln_w[:,None]*W) + ln_b @ W
    # The k-part of the qkv bias cancels in softmax; the v-part commutes
    # through the (row-stochastic) attention matrix into proj_b.
    qkv_eff = f64["qkv_w"] * f64["ln1_w"][:, None]
    qkv_bias = f64["ln1_b"] @ f64["qkv_w"]        # [3072]
    q_bias = qkv_bias[0:C]
    v_bias = qkv_bias[2 * C:3 * C]
    proj_b_eff = f64["proj_b"] + v_bias @ f64["proj_w"]
    fc1_eff = f64["fc1_w"] * f64["ln2_w"][:, None]
    fc1_b_eff = f64["fc1_b"] + f64["ln2_b"] @ f64["fc1_w"]
    weights = {
        "qkv_w": qkv_eff, "q_bias": q_bias, "proj_w": f64["proj_w"],
        "proj_b": proj_b_eff, "fc1_w": fc1_eff, "fc1_b": fc1_b_eff,
        "fc2_w": f64["fc2_w"], "fc2_b": f64["fc2_b"],
    }
    weights = {k: np.ascontiguousarray(v, dtype=np.float32)
               for k, v in weights.items()}
    if ACT_BF16:
        import ml_dtypes
        for k in ("qkv_w", "proj_w", "fc1_w", "fc2_w"):
            weights[k] = weights[k].astype(ml_dtypes.bfloat16)
    maps = []
    for c in range(N_CORES):
        b, q = c // 4, c % 4
        m = dict(weights)
        m["xb"] = np.ascontiguousarray(x[b])
        m["xown"] = np.ascontiguousarray(x[b, q * T:(q + 1) * T])
        maps.append(m)
    return maps


def assemble(results):
    out = np.empty((B, N, C), dtype=np.float32)
    for c in range(N_CORES):
        b, q = c // 4, c % 4
        out[b, q * T:(q + 1) * T] = results[c]["y"]
    return out


def kernel(**inputs) -> np.ndarray:
    nc = build()
    res = run_bass_kernel_spmd(nc, make_in_maps(inputs), list(range(N_CORES)))
    return assemble(res.results)



# revision 36
# speedup vs baseline: 195.2489x; 195.2489x over previous
"""Trainium2 Bass kernel for a pre-norm transformer block (attention + MLP).

Problem: x:[2, 2048, 1024], 16 heads x 64, MLP hidden 4096, fp32.

Sharding: data parallel over tokens, zero collectives. The 4096 tokens are
split into 8 blocks of 512 (core c handles batch c//4, sequence block c%4).
Each core receives its batch's 2048 rows ROTATED so its own 512 tokens are
rows 0:512 (keeps the program SPMD-uniform); it recomputes K/V for the whole
batch, runs attention for its 512 queries over all 2048 keys, then the MLP
for its own tokens. The host reassembles the output from 8 [512, 1024]
blocks.

Kernel structure (per core):
  - Fused phase 1: per 128-token block, LayerNorm stats (DVE bn_stats), the
    normalize runs on the Scalar/Act engine (per-partition scale=rsig,
    bias=-mu*rsig) emitting bf16, PE-transposes the block to channel-major
    (bf16, 1 cyc/row), drains to fp8e4, and immediately projects that
    block's K and V columns (all 16 heads) plus Q for own blocks.
  - Projections (q/k/v and attn-out) run in fp8e4 with DoubleRow perf mode
    (2 channel-blocks contracted per pass, 0.5 cyc/row): the accumulation
    over channel-block pairs maps directly onto DoubleRow's [p, 2, f]
    operand layout, so it is a pure reinterpretation of the bf16 tiling.
    Weights are pre-scaled x16 on the host (fp8e4 subnormal avoidance) and
    descaled in the PSUM drains. fc1/fc2 stay bf16: their error feeds the
    residual stream directly and fp8 there would blow the 2e-2 budget,
    whereas q/k errors cancel in softmax renormalization and v/proj errors
    are damped by the diffuse attention average (n_eff ~ 570) and the small
    attention-output magnitude (rms 0.064).
  - Attention stays bf16: scores transposed S^T[m, t] = k^T.T @ q^T with two
    row-packed K=64 matmuls per head pair; exp on the Act engine; V carries
    a ones column so the attention-value matmul accumulates softmax
    denominators; the 1/sum normalization is applied to the 64-row head
    outputs (scaled x16 into fp8 oT for the DoubleRow attn projection).
  - MLP: fc1+gelu into gT, fc2 with 8 held PSUM accumulators + residual.
"""

import numpy as np
from contextlib import ExitStack

import concourse.bass as bass
import concourse.tile as tile
from concourse import mybir
from concourse.bass_utils import run_bass_kernel_spmd
from concourse.masks import make_identity

FP32 = mybir.dt.float32
BF16 = mybir.dt.bfloat16
FP8 = mybir.dt.float8e4
AF = mybir.ActivationFunctionType
ALU = mybir.AluOpType
DR = mybir.MatmulPerfMode.DoubleRow

N_CORES = 8
B, N, C, H, D, F = 2, 2048, 1024, 16, 64, 4096
T = 512            # tokens owned per core
M = 2048           # keys (full batch sequence)
EPS = 1e-5
SCALE = float(D) ** -0.5   # 0.125
WS = 16.0          # fp8 weight pre-scale
IWS = 1.0 / WS

CB = C // 128      # 8 channel blocks
TB = T // 128      # 4 own-token blocks
MI = M // 128      # 16 key 128-chunks
FB = F // 128      # 32 mlp hidden blocks

SAB_BUFS = 2
STOP_AFTER = ""    # debug: stop emission after a phase name
K_ENG = "v"        # k-drain engine: v=DVE, p=Pool, a=Act
V_ENG = "a"        # v-drain engine
Q_ENG = "a"        # q-drain engine
SC_ENG = "v"       # proj descale engine
W1_BUFS = 4
P1H_BUFS = 3


def _drain(nc, eng, out, in_, scale, bias_ap=None):
    """PSUM->SBUF drain with scale (+ optional per-partition bias) on a
    selectable engine: v=DVE, p=Pool/gpsimd, a=Scalar/Act."""
    if eng == "a":
        if bias_ap is None:
            nc.scalar.activation(out=out, in_=in_, func=AF.Copy, scale=scale)
        else:
            nc.scalar.activation(out=out, in_=in_, func=AF.Identity,
                                 bias=bias_ap, scale=scale)
    else:
        e = nc.vector if eng == "v" else nc.gpsimd
        if bias_ap is None:
            e.tensor_scalar(out=out, in0=in_, scalar1=scale, scalar2=None,
                            op0=ALU.mult)
        else:
            e.tensor_scalar(out=out, in0=in_, scalar1=scale, scalar2=bias_ap,
                            op0=ALU.mult, op1=ALU.add)


def _ln_stats(nc, work, x_sb, eps_sb):
    """rsig [128,1] and negmurs=-mu*rsig [128,1] for LN along free dim."""
    stats = work.tile([128, 2, 6], FP32, name="ln_stats")
    nc.vector.bn_stats(out=stats[:, 0, :], in_=x_sb[:, 0:512])
    nc.vector.bn_stats(out=stats[:, 1, :], in_=x_sb[:, 512:1024])
    mv = work.tile([128, 2], FP32, name="ln_mv")
    nc.vector.bn_aggr(out=mv, in_=stats)
    sd = work.tile([128, 1], FP32, name="ln_sd")
    nc.scalar.activation(out=sd, in_=mv[:, 1:2], func=AF.Sqrt, bias=eps_sb,
                         scale=1.0)
    rsig = work.tile([128, 1], FP32, name="ln_rsig")
    nc.vector.reciprocal(out=rsig, in_=sd)
    murs = work.tile([128, 1], FP32, name="ln_murs")
    nc.vector.tensor_mul(out=murs, in0=mv[:, 0:1], in1=rsig)
    negmurs = work.tile([128, 1], FP32, name="ln_negmurs")
    nc.vector.tensor_scalar(out=negmurs, in0=murs, scalar1=-1.0, scalar2=None,
                            op0=ALU.mult)
    return rsig, negmurs


def _emit(ctx: ExitStack, tc: tile.TileContext, io: dict):
    nc = tc.nc

    xb = io["xb"]          # [2048, 1024] rotated batch rows (own = 0:512)
    qkv_w = io["qkv_w"]    # [1024, 3072] fp8 (ln1_w folded, x16)
    proj_w = io["proj_w"]  # [1024, 1024] fp8 (x16)
    proj_b = io["proj_b"]  # [1024] fp32 (+ folded v bias)
    q_bias = io["q_bias"]  # [1024] folded ln1_b @ Wq
    fc1_w, fc1_b = io["fc1_w"], io["fc1_b"]   # bf16 / fp32 (ln2 folded)
    fc2_w, fc2_b = io["fc2_w"], io["fc2_b"]
    y = io["y"]            # [512, 1024] output

    xb_r = xb.rearrange("(mi p) c -> mi p c", p=128)          # [16, 128, 1024]
    xown_r = xb.rearrange("(tb p) c -> p tb c", p=128)        # view; tb<4 own
    qkv_r = qkv_w.rearrange("(cb p) o -> p cb o", p=128)      # [128, 8, 3072]
    proj_r = proj_w.rearrange("(cb p) o -> p cb o", p=128)    # [128, 8, 1024]
    fc1_r = fc1_w.rearrange("(cb p) f -> p cb f", p=128)      # [128, 8, 4096]
    fc2_r = fc2_w.rearrange("(fb p) c -> p fb c", p=128)      # [128, 32, 1024]
    y_r = y.rearrange("(tb p) c -> p tb c", p=128)            # [128, 4, 1024]

    # --- constants (live whole kernel) ---
    consts = ctx.enter_context(tc.tile_pool(name="consts", bufs=1))

    ident_f = consts.tile([128, 128], FP32)
    make_identity(nc, ident_f)
    ident = consts.tile([128, 128], BF16)
    nc.vector.tensor_copy(out=ident, in_=ident_f)
    ones_f = consts.tile([128, 128], FP32)
    nc.vector.memset(ones_f, 1.0)
    ones = consts.tile([128, 128], BF16)
    nc.vector.tensor_copy(out=ones, in_=ones_f)
    eps_sb = consts.tile([128, 1], FP32)
    nc.vector.memset(eps_sb, EPS)

    def load_vec_pcb(vec, nblk, name):
        t = consts.tile([128, nblk], FP32, name=name)
        nc.sync.dma_start(out=t, in_=vec.rearrange("(b p) -> p b", p=128))
        return t

    qb_sb = load_vec_pcb(q_bias, CB, "qb")
    fc1b_sb = load_vec_pcb(fc1_b, FB, "fc1b")

    def bcast_rows_pool(pool, vec, name):
        t = pool.tile([128, C], FP32, name=name)
        src = bass.AP(tensor=vec.tensor, offset=vec.offset,
                      ap=[[0, 128]] + vec.ap)
        nc.sync.dma_start(out=t, in_=src)
        return t

    # --- persistent attention operands + mid tensors ---
    p_big = ctx.enter_context(tc.tile_pool(name="p_big", bufs=1))
    kT = p_big.tile([128, CB, M], BF16)          # k channel-major, 16 heads
    vg = p_big.tile([128, MI, H, D + 1], BF16)   # v token-major + ones col
    qT = p_big.tile([128, CB, T], BF16)          # q channel-major (own)
    oT = p_big.tile([128, CB, T], FP8)           # attn out x16, fp8

    p_mid = ctx.enter_context(tc.tile_pool(name="p_mid", bufs=1))
    x2 = p_mid.tile([128, TB, C], FP32)          # residual after attention
    h2T = p_mid.tile([128, CB, T], BF16)         # LN2 out, channel-major

    # ones column of vg (all mi, all heads at free-offset D)
    ones_col = bass.AP(
        tensor=ones.tensor, offset=ones[:, 0:1].offset,
        ap=[ones.ap[0], [0, MI], [0, H], [1, 1]])
    nc.vector.tensor_copy(out=vg[:, :, :, D:D + 1], in_=ones_col)

    # ------------------------------------------------------------------
    # Phase 1 (fused): per 128-token block: LN -> transpose -> K/V (+Q own)
    # ------------------------------------------------------------------
    with (
        tc.tile_pool(name="p1_w", bufs=1) as p1w,
        tc.tile_pool(name="p1_work", bufs=W1_BUFS) as w1,
        tc.tile_pool(name="p1_h", bufs=P1H_BUFS) as p1h,
        tc.tile_pool(name="p1_pst", bufs=2, space="PSUM") as ps_t,
        tc.tile_pool(name="p1_psp", bufs=2, space="PSUM") as ps_p,
    ):
        # first x blocks before the (bigger) weight DMAs so LN starts early
        xc_pre = []
        for mi in range(3):
            xc = w1.tile([128, C], FP32, name="p1_x")
            nc.sync.dma_start(out=xc, in_=xb_r[mi])
            xc_pre.append(xc)
        wq8 = p1w.tile([128, CB, C], FP8, name="wq8")
        nc.sync.dma_start(out=wq8, in_=qkv_r[:, :, 0:C])
        wk8 = p1w.tile([128, CB, C], FP8, name="wk8")
        nc.sync.dma_start(out=wk8, in_=qkv_r[:, :, C:2 * C])
        wv8 = p1w.tile([128, CB, C], FP8, name="wv8")
        nc.sync.dma_start(out=wv8, in_=qkv_r[:, :, 2 * C:3 * C])

        def emit_ln(mi):
            if mi < 3:
                xc = xc_pre[mi]
            else:
                xc = w1.tile([128, C], FP32, name="p1_x")
                nc.sync.dma_start(out=xc, in_=xb_r[mi])
            rsig, negmurs = _ln_stats(nc, w1, xc, eps_sb)
            xn = w1.tile([128, C], BF16, name="p1_xn")
            nc.scalar.activation(out=xn, in_=xc, func=AF.Identity,
                                 bias=negmurs, scale=rsig)
            hT8 = p1h.tile([128, CB, 128], FP8, name="hT8")
            for h4 in range(2):
                tp = ps_t.tile([128, 4, 128], BF16, name="p1_tp")
                for j in range(4):
                    cb = h4 * 4 + j
                    nc.tensor.transpose(
                        tp[:, j, :], xn[:, cb * 128:(cb + 1) * 128], ident)
                nc.vector.tensor_copy(
                    out=hT8[:, h4 * 4:(h4 + 1) * 4, :], in_=tp)
            return hT8

        # transposes run one block ahead of the projections so the PE
        # never waits on the Pool-engine hT8 drain
        hq = [emit_ln(0)]
        for mi in range(MI):
            if mi + 1 < MI:
                hq.append(emit_ln(mi + 1))
            hT8 = hq.pop(0)

            # K chunk: all 16 heads for this block, channel-major
            for half in range(2):
                pk = ps_p.tile([128, 4, 128], FP32, name="p1_pk")
                for kb4 in range(4):
                    kb = half * 4 + kb4
                    for j in range(4):
                        nc.tensor.matmul(
                            pk[:, kb4, :],
                            wk8[:, 2 * j:2 * j + 2, kb * 128:(kb + 1) * 128],
                            hT8[:, 2 * j:2 * j + 2, :],
                            start=(j == 0), stop=(j == 3), perf_mode=DR)
                kslc = slice(half * 4, (half + 1) * 4)
                _drain(nc, K_ENG, kT[:, kslc, mi * 128:(mi + 1) * 128],
                       pk, IWS)

            # V chunk: token-major [block, 16 heads x 64], + descale
            for half in range(2):
                pv = ps_p.tile([128, 512], FP32, name="p1_pv")
                for j in range(4):
                    nc.tensor.matmul(
                        pv, hT8[:, 2 * j:2 * j + 2, :],
                        wv8[:, 2 * j:2 * j + 2,
                            half * 512:(half + 1) * 512],
                        start=(j == 0), stop=(j == 3), perf_mode=DR)
                _drain(nc, V_ENG, vg[:, mi, half * 8:(half + 1) * 8, 0:D],
                       pv.rearrange("p (h d) -> p h d", d=D), IWS)

            # Q for own blocks (rotated: always blocks 0..3)
            if mi < TB:
                for half in range(2):
                    pq = ps_p.tile([128, 4, 128], FP32, name="p1_pq")
                    for qb4 in range(4):
                        qb = half * 4 + qb4
                        for j in range(4):
                            nc.tensor.matmul(
                                pq[:, qb4, :],
                                wq8[:, 2 * j:2 * j + 2,
                                    qb * 128:(qb + 1) * 128],
                                hT8[:, 2 * j:2 * j + 2, :],
                                start=(j == 0), stop=(j == 3), perf_mode=DR)
                    for qb4 in range(4):
                        qb = half * 4 + qb4
                        _drain(nc, Q_ENG,
                               qT[:, qb, mi * 128:(mi + 1) * 128],
                               pq[:, qb4, :], IWS, qb_sb[:, qb:qb + 1])

    if STOP_AFTER == "ln1":
        return

    # ------------------------------------------------------------------
    # Phase 2: attention, 8 head pairs, scores over 16 key chunks.
    # Unit = (pair, mi, head): one score matmul -> exp -> one AV matmul.
    # Scores are emitted 2 units ahead of the AV consumer so the Act
    # engine's exp latency never stalls the PE.
    # ------------------------------------------------------------------
    pjw = ctx.enter_context(tc.tile_pool(name="pj_pre", bufs=1))
    # prefetch the proj-phase operands so their DMAs run under attention
    wpj = pjw.tile([128, CB, C], FP8, name="wpj")
    nc.sync.dma_start(out=wpj, in_=proj_r)
    b1bc = bcast_rows_pool(pjw, proj_b, "b1bc")
    xob = pjw.tile([128, TB, C], FP32, name="xob")
    nc.sync.dma_start(out=xob, in_=xown_r[:, 0:TB, :])
    for tb in range(TB):
        nc.vector.tensor_add(out=xob[:, tb, :], in0=xob[:, tb, :],
                             in1=b1bc)

    with (
        tc.tile_pool(name="a_p", bufs=3) as pp,
        tc.tile_pool(name="a_r", bufs=2) as pr,
        tc.tile_pool(name="a_dram", bufs=2, space="DRAM") as p_dram,
        tc.tile_pool(name="a_pss", bufs=3, space="PSUM") as ps_s,
        tc.tile_pool(name="a_pso", bufs=1, space="PSUM") as ps_o,
    ):
        NU = (H // 2) * MI     # units: (pair, mi)

        def emit_scores(u):
            pair, mi = divmod(u, MI)
            msl = slice(mi * 128, (mi + 1) * 128)
            sAB = ps_s.tile([128, 2, T], FP32, name="sAB")
            nc.tensor.matmul(
                sAB[:, 0, :], kT[0:64, pair, msl],
                qT[0:64, pair, :], start=True, stop=True)
            nc.tensor.matmul(
                sAB[:, 1, :], kT[64:128, pair, msl],
                qT[64:128, pair, :], start=True, stop=True)
            return sAB

        def norm_head(o_sb, hh, pair):
            """o_sb: [65, T] SBUF copy (row 64 = softmax sums)."""
            rec = pr.tile([128, T], FP32, name=f"rec{hh}")
            nc.vector.reciprocal(out=rec[64:65, :], in_=o_sb[64:65, :])
            nc.vector.tensor_scalar(
                out=rec[64:65, :], in0=rec[64:65, :], scalar1=WS,
                scalar2=None, op0=ALU.mult)
            # partition-broadcast via DRAM bounce (stride-0 DRAM src)
            dr = p_dram.tile([T], FP32, name=f"dr{hh}")
            nc.sync.dma_start(out=dr, in_=rec[64:65, :])
            rb = pr.tile([128, T], FP32, name=f"rb{hh}")
            src = bass.AP(tensor=dr.tensor, offset=dr.offset,
                          ap=[[0, 64]] + dr.ap)
            nc.sync.dma_start(out=rb[0:64, :], in_=src)
            if hh == 0:
                nc.gpsimd.tensor_mul(
                    out=oT[0:64, pair, :], in0=o_sb[0:64, :],
                    in1=rb[0:64, :])
            else:
                tmpB = pr.tile([128, T], FP8, name="tmpB")
                nc.gpsimd.tensor_mul(
                    out=tmpB[0:64, :], in0=o_sb[0:64, :], in1=rb[0:64, :])
                nc.sync.dma_start(out=oT[64:128, pair, :], in_=tmpB[0:64, :])

        DEPTH = 2
        sq = [emit_scores(u) for u in range(DEPTH)]
        for u in range(NU):
            pair, mi = divmod(u, MI)
            if mi == 0:
                oA = ps_o.tile([128, T], FP32, name="oA")
                oB = ps_o.tile([128, T], FP32, name="oB")
            pAB = pp.tile([128, 2, T], BF16, name="pAB")
            nc.scalar.activation(out=pAB, in_=sq.pop(0), func=AF.Exp,
                                 scale=SCALE)
            if u + DEPTH < NU:
                sq.append(emit_scores(u + DEPTH))
            nc.tensor.matmul(
                oA[0:D + 1, :], vg[:, mi, 2 * pair, :], pAB[:, 0, :],
                start=(mi == 0), stop=(mi == MI - 1))
            nc.tensor.matmul(
                oB[0:D + 1, :], vg[:, mi, 2 * pair + 1, :], pAB[:, 1, :],
                start=(mi == 0), stop=(mi == MI - 1))
            if mi == MI - 1:
                # drain PSUM accumulators to SBUF on the (idle) Pool
                # engine so the next pair's AV matmuls reuse the banks
                # without waiting for the normalization chain
                oAc = pr.tile([128, T], FP32, name="oAc")
                nc.vector.tensor_copy(out=oAc[0:D + 1, :],
                                      in_=oA[0:D + 1, :])
                oBc = pr.tile([128, T], FP32, name="oBc")
                nc.vector.tensor_copy(out=oBc[0:D + 1, :],
                                      in_=oB[0:D + 1, :])
                norm_head(oAc, 0, pair)
                norm_head(oBc, 1, pair)

    if STOP_AFTER == "attn":
        return

    # ------------------------------------------------------------------
    # Phase 3+4: per token block: attn projection (fp8 DoubleRow) +
    # residual -> x2[tb] -> LN2 -> h2T[tb]; then x2 += fc2_b
    # ------------------------------------------------------------------
    b2bc = bcast_rows_pool(p_mid, fc2_b, "b2bc")
    with (
        tc.tile_pool(name="pj_work", bufs=3) as pjwork,
        tc.tile_pool(name="ln2_work", bufs=3) as w2,
        tc.tile_pool(name="pj_ps", bufs=2, space="PSUM") as ps_pj,
        tc.tile_pool(name="ln2_ps", bufs=2, space="PSUM") as ps_t2,
    ):
        def emit_ppj(tb):
            ppj = ps_pj.tile([128, 2, 512], FP32, name="ppj")
            for cc in range(2):
                for j in range(4):
                    nc.tensor.matmul(
                        ppj[:, cc, :],
                        oT[:, 2 * j:2 * j + 2, tb * 128:(tb + 1) * 128],
                        wpj[:, 2 * j:2 * j + 2, cc * 512:(cc + 1) * 512],
                        start=(j == 0), stop=(j == 3), perf_mode=DR)
            return ppj

        ppj_q = [emit_ppj(0)]
        for tb in range(TB):
            if tb + 1 < TB:
                ppj_q.append(emit_ppj(tb + 1))
            ppj = ppj_q.pop(0)
            sc = pjwork.tile([128, C], FP32, name="pj_sc")
            _drain(nc, SC_ENG, sc, ppj.rearrange("p a b -> p (a b)"),
                   1.0 / (WS * WS))
            nc.vector.tensor_add(out=x2[:, tb, :], in0=sc, in1=xob[:, tb, :])
            # LN2 for this block
            rsig, negmurs = _ln_stats(nc, w2, x2[:, tb, :], eps_sb)
            xn = w2.tile([128, C], BF16, name="ln2_xn")
            nc.scalar.activation(out=xn, in_=x2[:, tb, :], func=AF.Identity,
                                 bias=negmurs, scale=rsig)
            for h4 in range(2):
                tp = ps_t2.tile([128, 4, 128], BF16, name="ln2_tp")
                for j in range(4):
                    cb = h4 * 4 + j
                    nc.tensor.transpose(
                        tp[:, j, :], xn[:, cb * 128:(cb + 1) * 128], ident)
                nc.vector.tensor_copy(
                    out=h2T[:, h4 * 4:(h4 + 1) * 4, tb * 128:(tb + 1) * 128],
                    in_=tp)
            # fc2 residual base for this block (after LN2 consumed x2[tb])
            nc.vector.tensor_add(out=x2[:, tb, :], in0=x2[:, tb, :],
                                 in1=b2bc)

    if STOP_AFTER == "ln2":
        return

    # ------------------------------------------------------------------
    # Phase 5: MLP fc1 (gelu) -> gT [f, t]; fc2 + residual -> y
    # ------------------------------------------------------------------
    with (
        tc.tile_pool(name="p_g", bufs=1) as p_g,
        tc.tile_pool(name="f_w", bufs=2) as fw,
        tc.tile_pool(name="f_out", bufs=4) as fout,
    ):
        gT = p_g.tile([128, FB, T], BF16)

        with tc.tile_pool(name="f1_ps", bufs=3, space="PSUM") as ps_f1:
            for fq in range(FB // 4):
                w1t = fw.tile([128, CB, 512], BF16, name="w1t")
                nc.sync.dma_start(
                    out=w1t, in_=fc1_r[:, :, fq * 512:(fq + 1) * 512])
                for j in range(4):
                    fb = fq * 4 + j
                    pf = ps_f1.tile([128, T], FP32, name="pf")
                    for cb in range(CB):
                        nc.tensor.matmul(
                            pf, w1t[:, cb, j * 128:(j + 1) * 128],
                            h2T[:, cb, :],
                            start=(cb == 0), stop=(cb == CB - 1))
                    nc.scalar.activation(
                        out=gT[:, fb, :], in_=pf, func=AF.Gelu,
                        bias=fc1b_sb[:, fb:fb + 1], scale=1.0)

        # fc2: all 8 [t, c] psum accumulators live at once (8 banks), so
        # each weight tile streams exactly once at full DMA line width
        with tc.tile_pool(name="f2_ps", bufs=1, space="PSUM") as ps_f2:
            held = {}
            for tb in range(TB):
                for cc in range(2):
                    held[(tb, cc)] = ps_f2.tile(
                        [128, 512], FP32, name=f"pf2_{tb}_{cc}")
            for fb in range(FB):
                w2t = fw.tile([128, C], BF16, name="w2t")
                nc.sync.dma_start(out=w2t, in_=fc2_r[:, fb, :])
                for tb in range(TB):
                    for cc in range(2):
                        nc.tensor.matmul(
                            held[(tb, cc)], gT[:, fb, tb * 128:(tb + 1) * 128],
                            w2t[:, cc * 512:(cc + 1) * 512],
                            start=(fb == 0), stop=(fb == FB - 1))
            for tb in range(TB):
                for cc in range(2):
                    yt = fout.tile([128, 512], FP32, name="yt")
                    nc.vector.tensor_add(
                        out=yt, in0=held[(tb, cc)],
                        in1=x2[:, tb, cc * 512:(cc + 1) * 512])
                    nc.sync.dma_start(
                        out=y_r[:, tb, cc * 512:(cc + 1) * 512], in_=yt)


def split_excess_waits(nc, limit=1):
    """This walrus build only supports ONE sync wait per engine instruction.
    Move excess waits onto NOPs inserted just before the instruction on the
    same engine (for DMAs, move all waits so the descriptor carries none)."""
    for f in nc.m.functions:
        for bb in f.blocks:
            new_insts = []
            for inst in bb.instructions:
                si = getattr(inst, "sync_info", None)
                if si is not None and si.on_wait and len(si.on_wait) > limit:
                    waits = list(si.on_wait)
                    if isinstance(inst, mybir.InstDMACopy):
                        moved, si.on_wait = waits, []
                    else:
                        moved, si.on_wait = waits[limit:], waits[:limit]
                    for j, w in enumerate(moved):
                        nop = mybir.InstNoOp(
                            name=f"{inst.name}-xw{j}",
                            engine=inst.engine,
                            sync_info=mybir.SyncInfo(on_wait=[w], on_update=[]),
                            bass_nofuse=True,
                        )
                        new_insts.append(nop)
                new_insts.append(inst)
            bb.instructions[:] = new_insts


_CACHE = {}


def build():
    key = (SAB_BUFS, STOP_AFTER, K_ENG, V_ENG, Q_ENG, SC_ENG, W1_BUFS, P1H_BUFS)
    if key in _CACHE:
        return _CACHE[key]

    nc = bass.Bass("TRN2", target_bir_lowering=False, debug=False,
                   num_devices=N_CORES)
    io = {}
    io["xb"] = nc.dram_tensor("xb", [M, C], FP32, kind="ExternalInput").ap()
    io["qkv_w"] = nc.dram_tensor("qkv_w", [C, 3 * C], FP8,
                                 kind="ExternalInput").ap()
    io["proj_w"] = nc.dram_tensor("proj_w", [C, C], FP8,
                                  kind="ExternalInput").ap()
    io["proj_b"] = nc.dram_tensor("proj_b", [C], FP32,
                                  kind="ExternalInput").ap()
    io["q_bias"] = nc.dram_tensor("q_bias", [C], FP32,
                                  kind="ExternalInput").ap()
    io["fc1_w"] = nc.dram_tensor("fc1_w", [C, F], BF16,
                                 kind="ExternalInput").ap()
    io["fc1_b"] = nc.dram_tensor("fc1_b", [F], FP32,
                                 kind="ExternalInput").ap()
    io["fc2_w"] = nc.dram_tensor("fc2_w", [F, C], BF16,
                                 kind="ExternalInput").ap()
    io["fc2_b"] = nc.dram_tensor("fc2_b", [C], FP32,
                                 kind="ExternalInput").ap()
    io["y"] = nc.dram_tensor("y", [T, C], FP32, kind="ExternalOutput").ap()

    with tile.TileContext(nc) as tc:
        with ExitStack() as ctx:
            _emit(ctx, tc, io)

    split_excess_waits(nc)
    _CACHE[key] = nc
    return nc


def make_in_maps(inputs):
    import ml_dtypes
    x = np.ascontiguousarray(np.asarray(inputs["x"]), dtype=np.float32)
    f64 = {k: np.asarray(inputs[k], dtype=np.float64)
           for k in ("qkv_w", "proj_w", "proj_b", "ln1_w", "ln1_b", "ln2_w",
                     "ln2_b", "fc1_w", "fc1_b", "fc2_w", "fc2_b")}
    # Fold LayerNorm affines into the weights (exact up to rounding):
    #   h = xn*ln_w + ln_b;  h @ W = xn @ (ln_w[:,None]*W) + ln_b @ W
    # The k-part of the qkv bias cancels in softmax; the v-part commutes
    # through the (row-stochastic) attention matrix into proj_b.
    qkv_eff = f64["qkv_w"] * f64["ln1_w"][:, None]
    qkv_bias = f64["ln1_b"] @ f64["qkv_w"]        # [3072]
    q_bias = qkv_bias[0:C]
    v_bias = qkv_bias[2 * C:3 * C]
    proj_b_eff = f64["proj_b"] + v_bias @ f64["proj_w"]
    fc1_eff = f64["fc1_w"] * f64["ln2_w"][:, None]
    fc1_b_eff = f64["fc1_b"] + f64["ln2_b"] @ f64["fc1_w"]
    weights = {
        "qkv_w": (qkv_eff * WS).astype(np.float32).astype(
            ml_dtypes.float8_e4m3),
        "proj_w": (f64["proj_w"] * WS).astype(np.float32).astype(
            ml_dtypes.float8_e4m3),
        "q_bias": q_bias.astype(np.float32),
        "proj_b": proj_b_eff.astype(np.float32),
        "fc1_w": fc1_eff.astype(np.float32).astype(ml_dtypes.bfloat16),
        "fc1_b": fc1_b_eff.astype(np.float32),
        "fc2_w": f64["fc2_w"].astype(np.float32).astype(ml_dtypes.bfloat16),
        "fc2_b": f64["fc2_b"].astype(np.float32),
    }
    weights = {k: np.ascontiguousarray(v) for k, v in weights.items()}
    maps = []
    for c in range(N_CORES):
        b, q = c // 4, c % 4
        m = dict(weights)
        # rotate so own tokens are rows 0:512 (SPMD-uniform program)
        m["xb"] = np.ascontiguousarray(
            np.roll(x[b], -q * T, axis=0))
        maps.append(m)
    return maps


def assemble(results):
    out = np.empty((B, N, C), dtype=np.float32)
    for c in range(N_CORES):
        b, q = c // 4, c % 4
        out[b, q * T:(q + 1) * T] = results[c]["y"]
    return out


def kernel(**inputs) -> np.ndarray:
    nc = build()
    res = run_bass_kernel_spmd(nc, make_in_maps(inputs), list(range(N_CORES)))
    return assemble(res.results)
